# revision 2
# baseline (speedup 1.0000x reference)
"""Multi-head attention forward on 8 Trainium2 NeuronCores (Bass/Tile).

Problem: B=2, S=2048, HIDDEN=2048, HEADS=16, D_K=128, fp32 I/O,
mask all-ones (eval). torch-Linear convention: y = x @ W.T.

Sharding (head + output-row parallel, two AllToAlls, no all-reduce):
  - core c (0..7) owns heads {2c, 2c+1} for BOTH batches.
  - Phase A (per batch): project Q,K into the transposed [d, s] layout
    (row-streaming, 8 psum accumulators); project V DIRECTLY into the
    natural [s, d] layout (lhsT = vT seq-chunk, rhs = W_v slice) in two
    half-row passes — no PE transposes at all.
  - Phase B (per batch, per (q-block, head)): scoresT tiles [k, q] via
    Kh-stationary matmuls into WIDE multi-bank PSUM regions (3 k-tiles
    side by side), one wide EXP activation per region (the ACT fixed
    cost of ~352 cycles amortizes 3x, keeping the scalar engine faster
    than the PE), PV accumulation in the transposed layout. Softmax
    denominators: running DVE adds of the wide exp tiles + column folds
    + one GPSIMD partition_all_reduce. Softmax without max-subtraction
    (scores are O(few); mathematically identical to the reference).
  - One AllToAll per batch (8 ranks, 1MB): A2A#0 hides under phase A of
    batch 1, A2A#1 under batch 0's output projection.
  - Phase D: out_chunk = concat_chunk @ W_o.T per batch; each core
    produces 256 output rows per batch.
Phase order: A0 B0 [a2a0] A1 B1 [a2a1] D0 D1 — the PE never waits on a
collective, and phase B's exp/softmax bookkeeping runs under the PE's
matmul stream (ACT per-iteration time ~8.6us vs PE ~8.4us).
Host side: pre-transpose/cast inputs to bf16, slice weights per core,
scatter-gather the per-core [512, 2048] fp32 chunks into the full
output.
"""

import math
from contextlib import ExitStack

import ml_dtypes
import numpy as np

import concourse.bass as bass
import concourse.bass_isa as bass_isa
import concourse.tile as tile
from concourse import bacc, mybir
from concourse.bass_utils import run_bass_kernel_spmd

BF16 = mybir.dt.bfloat16
F32 = mybir.dt.float32
NPBF16 = ml_dtypes.bfloat16

HIDDEN = 2048
HEADS = 16
D_K = 128
B = 2
N_CORES = 8
HPC = HEADS // N_CORES          # heads per core (2)
DPC = HPC * D_K                 # concat cols per core (256)
NHT = HIDDEN // 128             # 16 hidden-dim 128-tiles


def _mha_kernel(ctx: ExitStack, tc: tile.TileContext, aps: dict, S: int):
    nc = tc.nc
    NKT = S // 128                   # seq 128-tiles (16)
    SBLK = min(512, S)               # matmul moving-dim block
    NSB = S // SBLK                  # 4
    QBLK = SBLK
    NQB = NSB
    SCB = S // N_CORES               # per-batch output rows per core (256)
    OBLK = 512
    NOB = HIDDEN // OBLK
    NST = SCB // 128                 # 2
    scale = 1.0 / math.sqrt(D_K)
    # phase-B kt groups: (start_kt, count); alternating psum slots A/B
    GRP = [(0, 3), (3, 3), (6, 3), (9, 3), (12, 3), (15, 1)]

    qT, kT, vT = aps["qT"], aps["kT"], aps["vT"]   # per batch [HIDDEN, S]
    wqT, wkT, wvT = aps["wqT"], aps["wkT"], aps["wvT"]  # [128, NHT*DPC]
    woT = aps["woT"]                                # [128, NHT*HIDDEN]
    out = aps["out"]                                # [B*SCB, HIDDEN] f32
    a2a_in = aps["a2a_in"]                          # per batch [8*DPC, SCB]
    a2a_out = aps["a2a_out"]                        # per batch [8*DPC, SCB]

    # ---- resident weights (pre-tiled on host) ----
    w_pool = ctx.enter_context(tc.tile_pool(name="wqkv", bufs=1))
    wq_sb = w_pool.tile([128, NHT * DPC], BF16, tag="wq")
    wk_sb = w_pool.tile([128, NHT * DPC], BF16, tag="wk")
    wv_sb = w_pool.tile([128, NHT * DPC], BF16, tag="wv")
    wo_sb = w_pool.tile([128, NHT * HIDDEN], BF16, tag="wo")
    wq_ck = NHT * DPC // 4
    for ck in range(4):
        nc.sync.dma_start(out=wq_sb[:, ck * wq_ck:(ck + 1) * wq_ck],
                          in_=wqT[:, ck * wq_ck:(ck + 1) * wq_ck])

    # ---- resident projection outputs (both batches) ----
    proj_pool = ctx.enter_context(tc.tile_pool(name="proj", bufs=1))
    qh_sb = [proj_pool.tile([128, HPC * S], BF16, tag=f"qh{b}", name=f"qh{b}")
             for b in range(B)]
    kh_sb = [proj_pool.tile([128, HPC * S], BF16, tag=f"kh{b}", name=f"kh{b}")
             for b in range(B)]
    vh_sb = [proj_pool.tile([128, NKT * DPC], BF16, tag=f"vh{b}", name=f"vh{b}")
             for b in range(B)]

    def phase_a(b):
        with tc.tile_pool(name="psA", bufs=8, space="PSUM") as psA, \
             tc.tile_pool(name="xrow", bufs=3) as xrow_pool:
            # Q / K: transposed [d, s] layout, 8 psum accumulators each.
            for wi, (src, w_sb, dst) in enumerate(
                    ((qT[b], wq_sb, qh_sb[b]), (kT[b], wk_sb, kh_sb[b]))):
                ps = [psA.tile([128, SBLK], F32, tag="psA", name=f"ps{wi}_{i}")
                      for i in range(HPC * NSB)]
                for ht in range(NHT):
                    row = xrow_pool.tile([128, S], BF16, tag="xrow")
                    nc.sync.dma_start(out=row[:],
                                      in_=src[ht * 128:(ht + 1) * 128, :])
                    if b == 0 and wi == 0 and ht == 0:
                        nc.sync.dma_start(out=wk_sb[:], in_=wkT[:, :])
                        nc.sync.dma_start(out=wv_sb[:], in_=wvT[:, :])
                    for dt in range(HPC):
                        for sb in range(NSB):
                            nc.tensor.matmul(
                                ps[dt * NSB + sb][:],
                                lhsT=w_sb[:, ht * DPC + dt * 128:
                                          ht * DPC + (dt + 1) * 128],
                                rhs=row[:, sb * SBLK:(sb + 1) * SBLK],
                                start=(ht == 0), stop=(ht == NHT - 1))
                for dt in range(HPC):
                    for sb in range(NSB):
                        nc.vector.tensor_copy(
                            dst[:, dt * S + sb * SBLK: dt * S + (sb + 1) * SBLK],
                            ps[dt * NSB + sb][:])

            # V directly in natural [s, d] layout: lhsT = vT seq-chunk
            # [128h, 128s], rhs = W_v ht-block [128h, 256d].  Two
            # half-row passes so the 16 seq-chunk accumulators fit the
            # 8 psum banks.
            SH = S // 2
            for half in range(2):
                psv = [psA.tile([128, DPC], F32, tag="psA",
                                name=f"psv{half}_{i}") for i in range(8)]
                for ht in range(NHT):
                    vrow = xrow_pool.tile([128, SH], BF16, tag="vrow")
                    nc.sync.dma_start(
                        out=vrow[:],
                        in_=vT[b][ht * 128:(ht + 1) * 128,
                                  half * SH:(half + 1) * SH])
                    for sti in range(8):
                        nc.tensor.matmul(
                            psv[sti][:],
                            lhsT=vrow[:, sti * 128:(sti + 1) * 128],
                            rhs=wv_sb[:, ht * DPC:(ht + 1) * DPC],
                            start=(ht == 0), stop=(ht == NHT - 1))
                for sti in range(8):
                    st = half * 8 + sti
                    nc.vector.tensor_copy(
                        vh_sb[b][:, st * DPC:(st + 1) * DPC], psv[sti][:])

    def phase_b(b):
        with tc.tile_pool(name="pssA", bufs=1, space="PSUM") as psA_pool, \
             tc.tile_pool(name="pssB", bufs=1, space="PSUM") as psB_pool, \
             tc.tile_pool(name="pspv", bufs=2, space="PSUM") as pv_pool, \
             tc.tile_pool(name="es", bufs=4) as es_pool, \
             tc.tile_pool(name="acc", bufs=2) as acc_pool, \
             tc.tile_pool(name="fld", bufs=2) as fld_pool, \
             tc.tile_pool(name="rb", bufs=2) as rb_pool, \
             tc.tile_pool(name="ao", bufs=2) as ao_pool:
            for qb in range(NQB):
                for l in range(HPC):
                    rhs_q = qh_sb[b][:, l * S + qb * QBLK:
                                     l * S + (qb + 1) * QBLK]
                    pv = pv_pool.tile([128, QBLK], F32, tag="pv")
                    wides = [None] * len(GRP)
                    ess = [None] * len(GRP)
                    accs = [None] * 5

                    def emit_scores(g):
                        k0, n = GRP[g]
                        pool = psA_pool if g % 2 == 0 else psB_pool
                        w = pool.tile([128, 3 * QBLK], F32, tag="w",
                                      name=f"wide{g}")
                        wides[g] = w
                        for j in range(n):
                            kt = k0 + j
                            nc.tensor.matmul(
                                w[:, j * QBLK:(j + 1) * QBLK],
                                lhsT=kh_sb[b][:, l * S + kt * 128:
                                              l * S + (kt + 1) * 128],
                                rhs=rhs_q, start=True, stop=True)

                    def emit_act(g):
                        n = GRP[g][1]
                        es = es_pool.tile([128, 3 * QBLK], BF16, tag="es",
                                          name=f"es{g}")
                        ess[g] = es
                        nc.scalar.activation(
                            es[:, :n * QBLK], wides[g][:, :n * QBLK],
                            mybir.ActivationFunctionType.Exp, scale=scale)

                    def emit_pv(g):
                        k0, n = GRP[g]
                        for j in range(n):
                            kt = k0 + j
                            nc.tensor.matmul(
                                pv[:],
                                lhsT=vh_sb[b][:, kt * DPC + l * 128:
                                              kt * DPC + (l + 1) * 128],
                                rhs=ess[g][:, j * QBLK:(j + 1) * QBLK],
                                start=(kt == 0), stop=(kt == NKT - 1))

                    def emit_add(g):
                        a = acc_pool.tile([128, 3 * QBLK], BF16, tag="acc",
                                          name=f"acc{g}")
                        if g == 1:
                            nc.vector.tensor_add(a[:], ess[0][:], ess[1][:])
                        else:
                            nc.vector.tensor_add(a[:], accs[g - 1][:],
                                                 ess[g][:])
                        accs[g] = a

                    # software-pipelined emission (1-group lookahead)
                    emit_scores(0)
                    emit_scores(1)
                    emit_act(0)
                    emit_pv(0)
                    for g in range(1, 5):
                        emit_scores(g + 1)
                        emit_act(g)
                        emit_pv(g)
                        emit_add(g)
                    emit_act(5)
                    emit_pv(5)

                    # denominator: fold 1536 -> 512 (+ last group), then
                    # partition reduce + reciprocal + normalize.
                    a4 = accs[4]
                    f1 = fld_pool.tile([128, QBLK], BF16, tag="f1")
                    nc.vector.tensor_add(f1[:], a4[:, 0:QBLK],
                                         a4[:, QBLK:2 * QBLK])
                    f2 = fld_pool.tile([128, QBLK], BF16, tag="f2")
                    nc.vector.tensor_add(f2[:], f1[:], a4[:, 2 * QBLK:3 * QBLK])
                    f3 = fld_pool.tile([128, QBLK], F32, tag="f3")
                    nc.vector.tensor_add(f3[:], f2[:], ess[5][:, 0:QBLK])
                    rb = rb_pool.tile([128, QBLK], F32, tag="rb")
                    nc.gpsimd.partition_all_reduce(
                        rb[:], f3[:], channels=128,
                        reduce_op=bass_isa.ReduceOp.add)
                    nc.vector.reciprocal_approx_fast(rb[:], rb[:])
                    ao = ao_pool.tile([128, QBLK], BF16, tag="ao")
                    nc.vector.tensor_mul(ao[:], pv[:], rb[:])
                    # scatter into a2a_in[b]: chunk m rows [m*DPC+l*128, +128)
                    q0 = qb * QBLK
                    while q0 < (qb + 1) * QBLK:
                        m = q0 // SCB
                        cend = min((qb + 1) * QBLK, (m + 1) * SCB)
                        nc.sync.dma_start(
                            out=a2a_in[b][m * DPC + l * 128:
                                          m * DPC + (l + 1) * 128,
                                          q0 - m * SCB: cend - m * SCB],
                            in_=ao[:, q0 - qb * QBLK: cend - qb * QBLK])
                        q0 = cend

    cc_tiles = []
    cc_pool = ctx.enter_context(tc.tile_pool(name="cc", bufs=B))

    def fire_a2a(b):
        coll = nc.gpsimd.collective_compute(
            "AllToAll", mybir.AluOpType.bypass,
            replica_groups=[list(range(N_CORES))],
            ins=[a2a_in[b][:, :]], outs=[a2a_out[b][:, :]])
        cc_sb = cc_pool.tile([128, NHT * SCB], BF16, tag="cc", name=f"cc{b}")
        dma = nc.sync.dma_start(
            out=cc_sb[:].rearrange("p (t s) -> p t s", t=NHT),
            in_=a2a_out[b][:, :].rearrange("(t p) s -> p t s", p=128))
        tile.add_dep_helper(dma.ins, coll.ins,
                            reason="a2a_out after collective")
        cc_tiles.append(cc_sb)

    phase_a(0)
    wo_ck = NHT * HIDDEN // 4
    for ck in range(4):
        nc.gpsimd.dma_start(out=wo_sb[:, ck * wo_ck:(ck + 1) * wo_ck],
                            in_=woT[:, ck * wo_ck:(ck + 1) * wo_ck])
    phase_b(0)
    fire_a2a(0)
    phase_a(1)
    phase_b(1)
    fire_a2a(1)

    # ================= Phase D: output projection (per batch) =========
    with tc.tile_pool(name="pso", bufs=8, space="PSUM") as pso_pool, \
         tc.tile_pool(name="osb", bufs=4) as osb_pool:
        for b in range(B):
            cc_sb = cc_tiles[b]
            for st in range(NST):
                pso = [pso_pool.tile([128, OBLK], F32, tag="pso",
                                     name=f"pso{b}_{st}_{i}")
                       for i in range(NOB)]
                for ht in range(NHT):
                    lhs = cc_sb[:, ht * SCB + st * 128:
                                ht * SCB + (st + 1) * 128]
                    for ot in range(NOB):
                        nc.tensor.matmul(
                            pso[ot][:], lhsT=lhs,
                            rhs=wo_sb[:, ht * HIDDEN + ot * OBLK:
                                      ht * HIDDEN + (ot + 1) * OBLK],
                            start=(ht == 0), stop=(ht == NHT - 1))
                for ot in range(NOB):
                    osb = osb_pool.tile([128, OBLK], F32, tag="osb")
                    nc.vector.tensor_copy(osb[:], pso[ot][:])
                    nc.gpsimd.dma_start(
                        out=out[b * SCB + st * 128: b * SCB + (st + 1) * 128,
                                ot * OBLK:(ot + 1) * OBLK],
                        in_=osb[:])


def build_nc(S: int):
    nc = bacc.Bacc("TRN2", target_bir_lowering=False, debug=False,
                   enable_asserts=False, num_devices=N_CORES)
    SCB = S // N_CORES
    aps = {
        "qT": [nc.dram_tensor(f"qT{b}", [HIDDEN, S], BF16,
                              kind="ExternalInput").ap() for b in range(B)],
        "kT": [nc.dram_tensor(f"kT{b}", [HIDDEN, S], BF16,
                              kind="ExternalInput").ap() for b in range(B)],
        "vT": [nc.dram_tensor(f"vT{b}", [HIDDEN, S], BF16,
                              kind="ExternalInput").ap() for b in range(B)],
        "wqT": nc.dram_tensor("wqT", [128, NHT * DPC], BF16,
                              kind="ExternalInput").ap(),
        "wkT": nc.dram_tensor("wkT", [128, NHT * DPC], BF16,
                              kind="ExternalInput").ap(),
        "wvT": nc.dram_tensor("wvT", [128, NHT * DPC], BF16,
                              kind="ExternalInput").ap(),
        "woT": nc.dram_tensor("woT", [128, NHT * HIDDEN], BF16,
                              kind="ExternalInput").ap(),
        "out": nc.dram_tensor("out", [B * SCB, HIDDEN], F32,
                              kind="ExternalOutput").ap(),
        "a2a_in": [nc.dram_tensor(f"a2a_in{b}", [N_CORES * DPC, SCB],
                                  BF16).ap() for b in range(B)],
        "a2a_out": [nc.dram_tensor(f"a2a_out{b}", [N_CORES * DPC, SCB],
                                   BF16).ap() for b in range(B)],
    }
    with tile.TileContext(nc) as tc:
        with ExitStack() as ctx:
            _mha_kernel(ctx, tc, aps, S)
    nc.compile()
    return nc


_NC_CACHE: dict = {}


def _tile_weight(w_slice_T):
    """[H, D] -> [128, (H//128)*D] with 128-row tiles laid out consecutively."""
    H, D = w_slice_T.shape
    return np.ascontiguousarray(
        w_slice_T.reshape(H // 128, 128, D).transpose(1, 0, 2).reshape(
            128, (H // 128) * D))


def make_in_maps(q, k, v, w_q, w_k, w_v, w_o):
    """Host-side shard/cast. Returns per-core input dicts."""
    qT = [np.ascontiguousarray(q[b].T).astype(NPBF16) for b in range(B)]
    kT = [np.ascontiguousarray(k[b].T).astype(NPBF16) for b in range(B)]
    vT = [np.ascontiguousarray(v[b].T).astype(NPBF16) for b in range(B)]
    woT = _tile_weight(np.ascontiguousarray(w_o.T).astype(NPBF16))
    in_maps = []
    for c in range(N_CORES):
        d0 = c * DPC
        m = {}
        for b in range(B):
            m[f"qT{b}"] = qT[b]
            m[f"kT{b}"] = kT[b]
            m[f"vT{b}"] = vT[b]
        m["wqT"] = _tile_weight(
            np.ascontiguousarray(w_q[d0:d0 + DPC, :].T).astype(NPBF16))
        m["wkT"] = _tile_weight(
            np.ascontiguousarray(w_k[d0:d0 + DPC, :].T).astype(NPBF16))
        m["wvT"] = _tile_weight(
            np.ascontiguousarray(w_v[d0:d0 + DPC, :].T).astype(NPBF16))
        m["woT"] = woT
        in_maps.append(m)
    return in_maps


def kernel(q, k, v, mask, w_q, w_k, w_v, w_o, _trace=False):
    q = np.asarray(q, np.float32)
    k = np.asarray(k, np.float32)
    v = np.asarray(v, np.float32)
    mask = np.asarray(mask)
    w_q = np.asarray(w_q, np.float32)
    w_k = np.asarray(w_k, np.float32)
    w_v = np.asarray(w_v, np.float32)
    w_o = np.asarray(w_o, np.float32)
    S = q.shape[1]

    if not np.all(mask != 0):
        # General-mask fallback (never hit for the eval problem: mask is
        # all ones).  Computed on host for correctness.
        return _numpy_reference(q, k, v, mask, w_q, w_k, w_v, w_o)

    if S not in _NC_CACHE:
        _NC_CACHE[S] = build_nc(S)
    nc = _NC_CACHE[S]

    in_maps = make_in_maps(q, k, v, w_q, w_k, w_v, w_o)
    res = run_bass_kernel_spmd(nc, in_maps, core_ids=list(range(N_CORES)),
                               trace=_trace)

    SCB = S // N_CORES
    out = np.empty((B, S, HIDDEN), np.float32)
    for c in range(N_CORES):
        for b in range(B):
            out[b, c * SCB:(c + 1) * SCB, :] = \
                res.results[c]["out"][b * SCB:(b + 1) * SCB, :]
    if _trace:
        return out, res
    return out


def _numpy_reference(q, k, v, mask, w_q, w_k, w_v, w_o):
    Bn, S, H = q.shape
    dk = H // HEADS

    def split_heads(x, w):
        y = x @ w.T
        return y.reshape(Bn, S, HEADS, dk).transpose(0, 2, 1, 3)

    qh = split_heads(q, w_q)
    kh = split_heads(k, w_k)
    vh = split_heads(v, w_v)
    s = np.einsum("bhqd,bhkd->bhqk", qh, kh) / np.sqrt(np.float32(dk))
    s = np.where(mask[:, None, :, :] == 0, np.float32(-1e9), s)
    s = s - s.max(-1, keepdims=True)
    e = np.exp(s)
    a = e / e.sum(-1, keepdims=True)
    o = np.einsum("bhqk,bhkd->bhqd", a, vh)
    o = o.transpose(0, 2, 1, 3).reshape(Bn, S, H)
    return (o @ w_o.T).astype(np.float32)


# revision 8
# speedup vs baseline: 1.0294x; 1.0294x over previous
"""Multi-head attention forward on 8 Trainium2 NeuronCores (Bass/Tile).

Problem: B=2, S=2048, HIDDEN=2048, HEADS=16, D_K=128, fp32 I/O,
mask all-ones (eval). torch-Linear convention: y = x @ W.T.

Sharding (head + output-row parallel, two AllToAlls, no all-reduce):
  - core c (0..7) owns heads {2c, 2c+1} for BOTH batches.
  - Phase A (per batch): project Q,K into the transposed [d, s] layout
    (row-streaming, 8 psum accumulators); project V DIRECTLY into the
    natural [s, d] layout (lhsT = vT seq-chunk, rhs = W_v slice) in two
    half-row passes — no PE transposes at all.
  - Phase B (per batch, per (q-block, head)): scoresT tiles [k, q] via
    Kh-stationary matmuls into WIDE multi-bank PSUM regions (3 k-tiles
    side by side), one wide EXP activation per region (the ACT fixed
    cost of ~352 cycles amortizes 3x, keeping the scalar engine faster
    than the PE), PV accumulation in the transposed layout. Softmax
    denominators: running DVE adds of the wide exp tiles + column folds
    + one GPSIMD partition_all_reduce. Softmax without max-subtraction
    (scores are O(few); mathematically identical to the reference).
  - One AllToAll per batch (8 ranks, 1MB): A2A#0 hides under phase A of
    batch 1, A2A#1 under batch 0's output projection.
  - Phase D: out_chunk = concat_chunk @ W_o.T per batch; each core
    produces 256 output rows per batch.
Phase order: A0 B0 [a2a0] A1 B1 [a2a1] D0 D1 — the PE never waits on a
collective, and phase B's exp/softmax bookkeeping runs under the PE's
matmul stream (ACT per-iteration time ~8.6us vs PE ~8.4us).
Host side: pre-transpose/cast inputs to bf16, slice weights per core,
scatter-gather the per-core [512, 2048] fp32 chunks into the full
output.
"""

import math
from contextlib import ExitStack

import ml_dtypes
import numpy as np

import concourse.bass as bass
import concourse.bass_isa as bass_isa
import concourse.tile as tile
from concourse import bacc, mybir
from concourse.bass_utils import run_bass_kernel_spmd

BF16 = mybir.dt.bfloat16
F32 = mybir.dt.float32
NPBF16 = ml_dtypes.bfloat16

HIDDEN = 2048
HEADS = 16
D_K = 128
B = 2
N_CORES = 8
HPC = HEADS // N_CORES          # heads per core (2)
DPC = HPC * D_K                 # concat cols per core (256)
NHT = HIDDEN // 128             # 16 hidden-dim 128-tiles


def _mha_kernel(ctx: ExitStack, tc: tile.TileContext, aps: dict, S: int):
    nc = tc.nc
    NKT = S // 128                   # seq 128-tiles (16)
    SBLK = min(512, S)               # matmul moving-dim block
    NSB = S // SBLK                  # 4
    QBLK = SBLK
    NQB = NSB
    SCB = S // N_CORES               # per-batch output rows per core (256)
    OBLK = 512
    NOB = HIDDEN // OBLK
    NST = SCB // 128                 # 2
    scale = 1.0 / math.sqrt(D_K)
    # phase-B kt groups: (start_kt, count); alternating psum slots A/B
    GRP = [(0, 3), (3, 3), (6, 3), (9, 3), (12, 3), (15, 1)]

    qT, kT, vT = aps["qT"], aps["kT"], aps["vT"]   # per batch [HIDDEN, S]
    wqT, wkT, wvT = aps["wqT"], aps["wkT"], aps["wvT"]  # [128, NHT*DPC]
    woT = aps["woT"]                                # [128, NHT*HIDDEN]
    out = aps["out"]                                # [B*SCB, HIDDEN] f32
    a2a_in = aps["a2a_in"]                          # per batch [8*DPC, SCB]
    a2a_out = aps["a2a_out"]                        # per batch [8*DPC, SCB]

    # ---- resident weights (pre-tiled on host) ----
    w_pool = ctx.enter_context(tc.tile_pool(name="wqkv", bufs=1))
    wq_sb = w_pool.tile([128, NHT * DPC], BF16, tag="wq")
    wk_sb = w_pool.tile([128, NHT * DPC], BF16, tag="wk")
    wv_sb = w_pool.tile([128, NHT * DPC], BF16, tag="wv")
    wo_sb = w_pool.tile([128, NHT * HIDDEN], BF16, tag="wo")

    # ---- projection outputs: batch 1 reuses batch 0's slots ----
    proj_pool = ctx.enter_context(tc.tile_pool(name="proj", bufs=1))

    # ---- persistent SBUF pools (ctx scope, no cross-phase space WAR) ----
    xrow_pool = ctx.enter_context(tc.tile_pool(name="xrow", bufs=5))
    vrow_pool = ctx.enter_context(tc.tile_pool(name="vrow", bufs=3))
    es_pool = ctx.enter_context(tc.tile_pool(name="es", bufs=7))
    acc_pool = ctx.enter_context(tc.tile_pool(name="acc", bufs=2))
    fld_pool = ctx.enter_context(tc.tile_pool(name="fld", bufs=2))
    rb_pool = ctx.enter_context(tc.tile_pool(name="rb", bufs=2))
    ao_pool = ctx.enter_context(tc.tile_pool(name="ao", bufs=2))
    osb_pool = ctx.enter_context(tc.tile_pool(name="osb", bufs=3))

    qh_sb = [None] * B
    kh_sb = [None] * B
    vh_sb = [None] * B

    def phase_a(b):
        qh_sb[b] = proj_pool.tile([128, HPC * S], BF16, tag="qh",
                                  name=f"qh{b}")
        kh_sb[b] = proj_pool.tile([128, HPC * S], BF16, tag="kh",
                                  name=f"kh{b}")
        vh_sb[b] = proj_pool.tile([128, NKT * DPC], BF16, tag="vh",
                                  name=f"vh{b}")
        with tc.tile_pool(name="psA", bufs=8, space="PSUM") as psA:
            # Q / K: transposed [d, s] layout, 8 psum accumulators each.
            for wi, (src, w_sb, dst) in enumerate(
                    ((qT[b], wq_sb, qh_sb[b]), (kT[b], wk_sb, kh_sb[b]))):
                ps = [psA.tile([128, SBLK], F32, tag="psA", name=f"ps{wi}_{i}")
                      for i in range(HPC * NSB)]
                if b == 0 and wi == 1:
                    # wv just before the V pass needs it; wk fired mid-Q.
                    nc.scalar.dma_start(out=wv_sb[:], in_=wvT[:, :])
                for ht in range(NHT):
                    row = xrow_pool.tile([128, S], BF16, tag="xrow")
                    nc.sync.dma_start(out=row[:],
                                      in_=src[ht * 128:(ht + 1) * 128, :])
                    if b == 0 and wi == 0 and ht == 8:
                        nc.scalar.dma_start(out=wk_sb[:], in_=wkT[:, :])
                    for dt in range(HPC):
                        for sb in range(NSB):
                            nc.tensor.matmul(
                                ps[dt * NSB + sb][:],
                                lhsT=w_sb[:, ht * DPC + dt * 128:
                                          ht * DPC + (dt + 1) * 128],
                                rhs=row[:, sb * SBLK:(sb + 1) * SBLK],
                                start=(ht == 0), stop=(ht == NHT - 1))
                for dt in range(HPC):
                    for sb in range(NSB):
                        nc.vector.tensor_copy(
                            dst[:, dt * S + sb * SBLK: dt * S + (sb + 1) * SBLK],
                            ps[dt * NSB + sb][:])

            # V directly in natural [s, d] layout: lhsT = vT seq-chunk
            # [128h, 128s], rhs = W_v ht-block [128h, 256d].  Two
            # half-row passes so the 16 seq-chunk accumulators fit the
            # 8 psum banks.
            SH = S // 2
            for half in range(2):
                psv = [psA.tile([128, DPC], F32, tag="psA",
                                name=f"psv{half}_{i}") for i in range(8)]
                for ht in range(NHT):
                    vrow = vrow_pool.tile([128, SH], BF16, tag="vrow")
                    nc.sync.dma_start(
                        out=vrow[:],
                        in_=vT[b][ht * 128:(ht + 1) * 128,
                                  half * SH:(half + 1) * SH])
                    for sti in range(8):
                        nc.tensor.matmul(
                            psv[sti][:],
                            lhsT=vrow[:, sti * 128:(sti + 1) * 128],
                            rhs=wv_sb[:, ht * DPC:(ht + 1) * DPC],
                            start=(ht == 0), stop=(ht == NHT - 1))
                for sti in range(8):
                    st = half * 8 + sti
                    nc.vector.tensor_copy(
                        vh_sb[b][:, st * DPC:(st + 1) * DPC], psv[sti][:])

    def phase_b(b):
        with tc.tile_pool(name="pssA", bufs=1, space="PSUM") as psA_pool, \
             tc.tile_pool(name="pssB", bufs=1, space="PSUM") as psB_pool, \
             tc.tile_pool(name="pspv", bufs=2, space="PSUM") as pv_pool:
            for qb in range(NQB):
                for l in range(HPC):
                    rhs_q = qh_sb[b][:, l * S + qb * QBLK:
                                     l * S + (qb + 1) * QBLK]
                    pv = pv_pool.tile([128, QBLK], F32, tag="pv")
                    wides = [None] * len(GRP)
                    ess = [None] * len(GRP)
                    accs = [None] * 5

                    def emit_scores(g):
                        k0, n = GRP[g]
                        pool = psA_pool if g % 2 == 0 else psB_pool
                        w = pool.tile([128, 3 * QBLK], F32, tag="w",
                                      name=f"wide{g}")
                        wides[g] = w
                        for j in range(n):
                            kt = k0 + j
                            nc.tensor.matmul(
                                w[:, j * QBLK:(j + 1) * QBLK],
                                lhsT=kh_sb[b][:, l * S + kt * 128:
                                              l * S + (kt + 1) * 128],
                                rhs=rhs_q, start=True, stop=True)

                    def emit_act(g):
                        n = GRP[g][1]
                        es = es_pool.tile([128, 3 * QBLK], BF16, tag="es",
                                          name=f"es{g}")
                        ess[g] = es
                        nc.scalar.activation(
                            es[:, :n * QBLK], wides[g][:, :n * QBLK],
                            mybir.ActivationFunctionType.Exp, scale=scale)

                    def emit_pv(g):
                        k0, n = GRP[g]
                        for j in range(n):
                            kt = k0 + j
                            nc.tensor.matmul(
                                pv[:],
                                lhsT=vh_sb[b][:, kt * DPC + l * 128:
                                              kt * DPC + (l + 1) * 128],
                                rhs=ess[g][:, j * QBLK:(j + 1) * QBLK],
                                start=(kt == 0), stop=(kt == NKT - 1))

                    def emit_add(g):
                        a = acc_pool.tile([128, 3 * QBLK], BF16, tag="acc",
                                          name=f"acc{g}")
                        if g == 1:
                            nc.vector.tensor_add(a[:], ess[0][:], ess[1][:])
                        else:
                            nc.vector.tensor_add(a[:], accs[g - 1][:],
                                                 ess[g][:])
                        accs[g] = a

                    # software-pipelined emission (1-group lookahead)
                    emit_scores(0)
                    emit_scores(1)
                    emit_act(0)
                    emit_pv(0)
                    for g in range(1, 5):
                        emit_scores(g + 1)
                        emit_act(g)
                        emit_pv(g)
                        emit_add(g)
                    emit_act(5)
                    emit_pv(5)

                    # denominator: fold 1536 -> 512 (+ last group), then
                    # partition reduce + reciprocal + normalize.
                    a4 = accs[4]
                    f1 = fld_pool.tile([128, QBLK], BF16, tag="f1")
                    nc.vector.tensor_add(f1[:], a4[:, 0:QBLK],
                                         a4[:, QBLK:2 * QBLK])
                    f2 = fld_pool.tile([128, QBLK], BF16, tag="f2")
                    nc.vector.tensor_add(f2[:], f1[:], a4[:, 2 * QBLK:3 * QBLK])
                    f3 = fld_pool.tile([128, QBLK], F32, tag="f3")
                    nc.vector.tensor_add(f3[:], f2[:], ess[5][:, 0:QBLK])
                    rb = rb_pool.tile([128, QBLK], F32, tag="rb")
                    nc.gpsimd.partition_all_reduce(
                        rb[:], f3[:], channels=128,
                        reduce_op=bass_isa.ReduceOp.add)
                    nc.vector.reciprocal_approx_fast(rb[:], rb[:])
                    ao = ao_pool.tile([128, QBLK], BF16, tag="ao")
                    nc.vector.tensor_mul(ao[:], pv[:], rb[:])
                    # scatter into a2a_in[b]: chunk m rows [m*DPC+l*128, +128)
                    q0 = qb * QBLK
                    while q0 < (qb + 1) * QBLK:
                        m = q0 // SCB
                        cend = min((qb + 1) * QBLK, (m + 1) * SCB)
                        nc.sync.dma_start(
                            out=a2a_in[b][m * DPC + l * 128:
                                          m * DPC + (l + 1) * 128,
                                          q0 - m * SCB: cend - m * SCB],
                            in_=ao[:, q0 - qb * QBLK: cend - qb * QBLK])
                        q0 = cend

    cc_tiles = []
    cc_pool = ctx.enter_context(tc.tile_pool(name="cc", bufs=B))

    def fire_a2a(b):
        coll = nc.gpsimd.collective_compute(
            "AllToAll", mybir.AluOpType.bypass,
            replica_groups=[list(range(N_CORES))],
            ins=[a2a_in[b][:, :]], outs=[a2a_out[b][:, :]])
        cc_sb = cc_pool.tile([128, NHT * SCB], BF16, tag="cc", name=f"cc{b}")
        dma = nc.sync.dma_start(
            out=cc_sb[:].rearrange("p (t s) -> p t s", t=NHT),
            in_=a2a_out[b][:, :].rearrange("(t p) s -> p t s", p=128))
        tile.add_dep_helper(dma.ins, coll.ins,
                            reason="a2a_out after collective")
        cc_tiles.append(cc_sb)

    # weights go down the scalar engine's DMA queue so the sync queue
    # carries only input rows (rows must win the pre-barrier bandwidth).
    wq_ck = NHT * DPC // 4
    for ck in range(4):
        nc.scalar.dma_start(out=wq_sb[:, ck * wq_ck:(ck + 1) * wq_ck],
                            in_=wqT[:, ck * wq_ck:(ck + 1) * wq_ck])
    phase_a(0)
    wo_ck = NHT * HIDDEN // 4
    for ck in range(4):
        nc.gpsimd.dma_start(out=wo_sb[:, ck * wo_ck:(ck + 1) * wo_ck],
                            in_=woT[:, ck * wo_ck:(ck + 1) * wo_ck])
    phase_b(0)
    fire_a2a(0)
    phase_a(1)
    phase_b(1)
    fire_a2a(1)

    # ================= Phase D: output projection (per batch) =========
    with tc.tile_pool(name="pso", bufs=8, space="PSUM") as pso_pool:
        for b in range(B):
            cc_sb = cc_tiles[b]
            for st in range(NST):
                pso = [pso_pool.tile([128, OBLK], F32, tag="pso",
                                     name=f"pso{b}_{st}_{i}")
                       for i in range(NOB)]
                for ht in range(NHT):
                    lhs = cc_sb[:, ht * SCB + st * 128:
                                ht * SCB + (st + 1) * 128]
                    for ot in range(NOB):
                        nc.tensor.matmul(
                            pso[ot][:], lhsT=lhs,
                            rhs=wo_sb[:, ht * HIDDEN + ot * OBLK:
                                      ht * HIDDEN + (ot + 1) * OBLK],
                            start=(ht == 0), stop=(ht == NHT - 1))
                for ot in range(NOB):
                    osb = osb_pool.tile([128, OBLK], F32, tag="osb")
                    nc.vector.tensor_copy(osb[:], pso[ot][:])
                    nc.gpsimd.dma_start(
                        out=out[b * SCB + st * 128: b * SCB + (st + 1) * 128,
                                ot * OBLK:(ot + 1) * OBLK],
                        in_=osb[:])


def build_nc(S: int):
    nc = bacc.Bacc("TRN2", target_bir_lowering=False, debug=False,
                   enable_asserts=False, num_devices=N_CORES)
    SCB = S // N_CORES
    aps = {
        "qT": [nc.dram_tensor(f"qT{b}", [HIDDEN, S], BF16,
                              kind="ExternalInput").ap() for b in range(B)],
        "kT": [nc.dram_tensor(f"kT{b}", [HIDDEN, S], BF16,
                              kind="ExternalInput").ap() for b in range(B)],
        "vT": [nc.dram_tensor(f"vT{b}", [HIDDEN, S], BF16,
                              kind="ExternalInput").ap() for b in range(B)],
        "wqT": nc.dram_tensor("wqT", [128, NHT * DPC], BF16,
                              kind="ExternalInput").ap(),
        "wkT": nc.dram_tensor("wkT", [128, NHT * DPC], BF16,
                              kind="ExternalInput").ap(),
        "wvT": nc.dram_tensor("wvT", [128, NHT * DPC], BF16,
                              kind="ExternalInput").ap(),
        "woT": nc.dram_tensor("woT", [128, NHT * HIDDEN], BF16,
                              kind="ExternalInput").ap(),
        "out": nc.dram_tensor("out", [B * SCB, HIDDEN], F32,
                              kind="ExternalOutput").ap(),
        "a2a_in": [nc.dram_tensor(f"a2a_in{b}", [N_CORES * DPC, SCB],
                                  BF16).ap() for b in range(B)],
        "a2a_out": [nc.dram_tensor(f"a2a_out{b}", [N_CORES * DPC, SCB],
                                   BF16).ap() for b in range(B)],
    }
    with tile.TileContext(nc) as tc:
        with ExitStack() as ctx:
            _mha_kernel(ctx, tc, aps, S)
    nc.compile()
    return nc


_NC_CACHE: dict = {}


def _tile_weight(w_slice_T):
    """[H, D] -> [128, (H//128)*D] with 128-row tiles laid out consecutively."""
    H, D = w_slice_T.shape
    return np.ascontiguousarray(
        w_slice_T.reshape(H // 128, 128, D).transpose(1, 0, 2).reshape(
            128, (H // 128) * D))


def make_in_maps(q, k, v, w_q, w_k, w_v, w_o):
    """Host-side shard/cast. Returns per-core input dicts."""
    qT = [np.ascontiguousarray(q[b].T).astype(NPBF16) for b in range(B)]
    kT = [np.ascontiguousarray(k[b].T).astype(NPBF16) for b in range(B)]
    vT = [np.ascontiguousarray(v[b].T).astype(NPBF16) for b in range(B)]
    woT = _tile_weight(np.ascontiguousarray(w_o.T).astype(NPBF16))
    in_maps = []
    for c in range(N_CORES):
        d0 = c * DPC
        m = {}
        for b in range(B):
            m[f"qT{b}"] = qT[b]
            m[f"kT{b}"] = kT[b]
            m[f"vT{b}"] = vT[b]
        m["wqT"] = _tile_weight(
            np.ascontiguousarray(w_q[d0:d0 + DPC, :].T).astype(NPBF16))
        m["wkT"] = _tile_weight(
            np.ascontiguousarray(w_k[d0:d0 + DPC, :].T).astype(NPBF16))
        m["wvT"] = _tile_weight(
            np.ascontiguousarray(w_v[d0:d0 + DPC, :].T).astype(NPBF16))
        m["woT"] = woT
        in_maps.append(m)
    return in_maps


def kernel(q, k, v, mask, w_q, w_k, w_v, w_o, _trace=False):
    q = np.asarray(q, np.float32)
    k = np.asarray(k, np.float32)
    v = np.asarray(v, np.float32)
    mask = np.asarray(mask)
    w_q = np.asarray(w_q, np.float32)
    w_k = np.asarray(w_k, np.float32)
    w_v = np.asarray(w_v, np.float32)
    w_o = np.asarray(w_o, np.float32)
    S = q.shape[1]

    if not np.all(mask != 0):
        # General-mask fallback (never hit for the eval problem: mask is
        # all ones).  Computed on host for correctness.
        return _numpy_reference(q, k, v, mask, w_q, w_k, w_v, w_o)

    if S not in _NC_CACHE:
        _NC_CACHE[S] = build_nc(S)
    nc = _NC_CACHE[S]

    in_maps = make_in_maps(q, k, v, w_q, w_k, w_v, w_o)
    res = run_bass_kernel_spmd(nc, in_maps, core_ids=list(range(N_CORES)),
                               trace=_trace)

    SCB = S // N_CORES
    out = np.empty((B, S, HIDDEN), np.float32)
    for c in range(N_CORES):
        for b in range(B):
            out[b, c * SCB:(c + 1) * SCB, :] = \
                res.results[c]["out"][b * SCB:(b + 1) * SCB, :]
    if _trace:
        return out, res
    return out


def _numpy_reference(q, k, v, mask, w_q, w_k, w_v, w_o):
    Bn, S, H = q.shape
    dk = H // HEADS

    def split_heads(x, w):
        y = x @ w.T
        return y.reshape(Bn, S, HEADS, dk).transpose(0, 2, 1, 3)

    qh = split_heads(q, w_q)
    kh = split_heads(k, w_k)
    vh = split_heads(v, w_v)
    s = np.einsum("bhqd,bhkd->bhqk", qh, kh) / np.sqrt(np.float32(dk))
    s = np.where(mask[:, None, :, :] == 0, np.float32(-1e9), s)
    s = s - s.max(-1, keepdims=True)
    e = np.exp(s)
    a = e / e.sum(-1, keepdims=True)
    o = np.einsum("bhqk,bhkd->bhqd", a, vh)
    o = o.transpose(0, 2, 1, 3).reshape(Bn, S, H)
    return (o @ w_o.T).astype(np.float32)


# revision 16
# speedup vs baseline: 1.0808x; 1.0499x over previous
"""Multi-head attention forward on 8 Trainium2 NeuronCores (Bass/Tile).

Problem: B=2, S=2048, HIDDEN=2048, HEADS=16, D_K=128, fp32 I/O,
mask all-ones (eval). torch-Linear convention: y = x @ W.T.

Sharding (head + output-row parallel, two AllToAlls, no all-reduce):
  - core c (0..7) owns heads {2c, 2c+1} for BOTH batches.
  - Phase A (per batch): project Q,K into the transposed [d, s] layout
    (row-streaming, 8 psum accumulators); project V DIRECTLY into the
    natural [s, d] layout (lhsT = vT seq-chunk, rhs = W_v slice) in two
    half-row passes — no PE transposes at all.
  - Phase B (per batch, per (q-block, head)): scoresT tiles [k, q] via
    Kh-stationary matmuls into WIDE multi-bank PSUM regions (3 k-tiles
    side by side), one wide EXP activation per region (the ACT fixed
    cost of ~352 cycles amortizes 3x, keeping the scalar engine faster
    than the PE), PV accumulation in the transposed layout. Softmax
    denominators: running DVE adds of the wide exp tiles + column folds
    + one GPSIMD partition_all_reduce. Softmax without max-subtraction
    (scores are O(few); mathematically identical to the reference).
  - One AllToAll per batch (8 ranks, 1MB): A2A#0 hides under phase A of
    batch 1, A2A#1 under batch 0's output projection.
  - Phase D: out_chunk = concat_chunk @ W_o.T per batch; each core
    produces 256 output rows per batch.
Phase order: A0 B0 [a2a0] A1 B1 [a2a1] D0 D1 — the PE never waits on a
collective, and phase B's exp/softmax bookkeeping runs under the PE's
matmul stream (ACT per-iteration time ~8.6us vs PE ~8.4us).
Host side: pre-transpose/cast inputs to bf16, slice weights per core,
scatter-gather the per-core [512, 2048] fp32 chunks into the full
output.
"""

import math
from contextlib import ExitStack

import ml_dtypes
import numpy as np

import concourse.bass as bass
import concourse.bass_isa as bass_isa
import concourse.tile as tile
from concourse import bacc, mybir
from concourse.bass_utils import run_bass_kernel_spmd

BF16 = mybir.dt.bfloat16
F32 = mybir.dt.float32
NPBF16 = ml_dtypes.bfloat16

HIDDEN = 2048
HEADS = 16
D_K = 128
B = 2
N_CORES = 8
HPC = HEADS // N_CORES          # heads per core (2)
DPC = HPC * D_K                 # concat cols per core (256)
NHT = HIDDEN // 128             # 16 hidden-dim 128-tiles


def _mha_kernel(ctx: ExitStack, tc: tile.TileContext, aps: dict, S: int):
    nc = tc.nc
    NKT = S // 128                   # seq 128-tiles (16)
    SBLK = min(512, S)               # matmul moving-dim block
    NSB = S // SBLK                  # 4
    QBLK = SBLK
    NQB = NSB
    SCB = S // N_CORES               # per-batch output rows per core (256)
    OBLK = 512
    NOB = HIDDEN // OBLK
    NST = SCB // 128                 # 2
    scale = 1.0 / math.sqrt(D_K)
    # phase-B kt groups: (start_kt, count); alternating psum slots A/B
    GRP = [(0, 3), (3, 3), (6, 3), (9, 3), (12, 3), (15, 1)]

    qT, kT, vT = aps["qT"], aps["kT"], aps["vT"]   # per batch [HIDDEN, S]
    wqT, wkT, wvT = aps["wqT"], aps["wkT"], aps["wvT"]  # [128, NHT*DPC]
    woT = aps["woT"]                                # [128, NHT*HIDDEN]
    out = aps["out"]                                # [B*SCB, HIDDEN] f32
    a2a_in = aps["a2a_in"]                          # per batch [8*DPC, SCB]
    a2a_out = aps["a2a_out"]                        # per batch [8*DPC, SCB]

    # ---- resident weights (pre-tiled on host) ----
    w_pool = ctx.enter_context(tc.tile_pool(name="wqkv", bufs=1))
    wq_sb = w_pool.tile([128, NHT * DPC], BF16, tag="wq")
    wk_sb = w_pool.tile([128, NHT * DPC], BF16, tag="wk")
    wv_sb = w_pool.tile([128, NHT * DPC], BF16, tag="wv")
    wo_sb = w_pool.tile([128, NHT * HIDDEN], BF16, tag="wo")

    # ---- projection outputs: batch 1 reuses batch 0's slots ----
    proj_pool = ctx.enter_context(tc.tile_pool(name="proj", bufs=1))

    # ---- persistent SBUF pools (ctx scope, no cross-phase space WAR) ----
    xrow_pool = ctx.enter_context(tc.tile_pool(name="xrow", bufs=5))
    vrow_pool = ctx.enter_context(tc.tile_pool(name="vrow", bufs=3))
    es_pool = ctx.enter_context(tc.tile_pool(name="es", bufs=7))
    acc_pool = ctx.enter_context(tc.tile_pool(name="acc", bufs=2))
    fld_pool = ctx.enter_context(tc.tile_pool(name="fld", bufs=2))
    rb_pool = ctx.enter_context(tc.tile_pool(name="rb", bufs=2))
    ao_pool = ctx.enter_context(tc.tile_pool(name="ao", bufs=2))
    osb_pool = ctx.enter_context(tc.tile_pool(name="osb", bufs=3))

    qh_sb = [None] * B
    kh_sb = [None] * B
    vh_sb = [None] * B

    # all-ones stationary tile for the PE partition-reduce of the last
    # iteration's softmax denominator (short critical tail at phase ends)
    ones_pool = ctx.enter_context(tc.tile_pool(name="ones", bufs=1))
    ones_sb = ones_pool.tile([128, 128], BF16, tag="ones")
    nc.vector.memset(ones_sb[:], 1.0)

    def phase_a(b):
        qh_sb[b] = proj_pool.tile([128, HPC * S], BF16, tag="qh",
                                  name=f"qh{b}")
        kh_sb[b] = proj_pool.tile([128, HPC * S], BF16, tag="kh",
                                  name=f"kh{b}")
        vh_sb[b] = proj_pool.tile([128, NKT * DPC], BF16, tag="vh",
                                  name=f"vh{b}")
        with tc.tile_pool(name="psA", bufs=8, space="PSUM") as psA:
            # Q / K: transposed [d, s] layout, 8 psum accumulators each.
            for wi, (src, w_sb, dst) in enumerate(
                    ((qT[b], wq_sb, qh_sb[b]), (kT[b], wk_sb, kh_sb[b]))):
                ps = [psA.tile([128, SBLK], F32, tag="psA", name=f"ps{wi}_{i}")
                      for i in range(HPC * NSB)]
                for ht in range(NHT):
                    row = xrow_pool.tile([128, S], BF16, tag="xrow")
                    nc.sync.dma_start(out=row[:],
                                      in_=src[ht * 128:(ht + 1) * 128, :])
                    # weight DMAs ride the same sync queue, placed in the
                    # row stream just-in-time so they never compete with
                    # the rows the PE is about to consume.
                    if b == 0 and wi == 0 and ht == 12:
                        nc.sync.dma_start(out=wk_sb[:], in_=wkT[:, :])
                    if b == 0 and wi == 1 and ht == 4:
                        nc.sync.dma_start(out=wv_sb[:], in_=wvT[:, :])
                    for dt in range(HPC):
                        for sb in range(NSB):
                            nc.tensor.matmul(
                                ps[dt * NSB + sb][:],
                                lhsT=w_sb[:, ht * DPC + dt * 128:
                                          ht * DPC + (dt + 1) * 128],
                                rhs=row[:, sb * SBLK:(sb + 1) * SBLK],
                                start=(ht == 0), stop=(ht == NHT - 1))
                for dt in range(HPC):
                    for sb in range(NSB):
                        nc.vector.tensor_copy(
                            dst[:, dt * S + sb * SBLK: dt * S + (sb + 1) * SBLK],
                            ps[dt * NSB + sb][:])

            # V directly in natural [s, d] layout: lhsT = vT seq-chunk
            # [128h, 128s], rhs = W_v ht-block [128h, 256d].  Two
            # half-row passes so the 16 seq-chunk accumulators fit the
            # 8 psum banks.
            SH = S // 2
            for half in range(2):
                psv = [psA.tile([128, DPC], F32, tag="psA",
                                name=f"psv{half}_{i}") for i in range(8)]
                for ht in range(NHT):
                    vrow = vrow_pool.tile([128, SH], BF16, tag="vrow")
                    nc.sync.dma_start(
                        out=vrow[:],
                        in_=vT[b][ht * 128:(ht + 1) * 128,
                                  half * SH:(half + 1) * SH])
                    for sti in range(8):
                        nc.tensor.matmul(
                            psv[sti][:],
                            lhsT=vrow[:, sti * 128:(sti + 1) * 128],
                            rhs=wv_sb[:, ht * DPC:(ht + 1) * DPC],
                            start=(ht == 0), stop=(ht == NHT - 1))
                for sti in range(8):
                    st = half * 8 + sti
                    nc.vector.tensor_copy(
                        vh_sb[b][:, st * DPC:(st + 1) * DPC], psv[sti][:])

    def phase_b(b):
        with tc.tile_pool(name="pssA", bufs=1, space="PSUM") as psA_pool, \
             tc.tile_pool(name="pssB", bufs=1, space="PSUM") as psB_pool, \
             tc.tile_pool(name="pspv", bufs=2, space="PSUM") as pv_pool:
            for qb in range(NQB):
                for l in range(HPC):
                    rhs_q = qh_sb[b][:, l * S + qb * QBLK:
                                     l * S + (qb + 1) * QBLK]
                    pv = pv_pool.tile([128, QBLK], F32, tag="pv")
                    wides = [None] * len(GRP)
                    ess = [None] * len(GRP)
                    accs = [None] * 5

                    def emit_scores(g):
                        k0, n = GRP[g]
                        pool = psA_pool if g % 2 == 0 else psB_pool
                        w = pool.tile([128, 3 * QBLK], F32, tag="w",
                                      name=f"wide{g}")
                        wides[g] = w
                        for j in range(n):
                            kt = k0 + j
                            nc.tensor.matmul(
                                w[:, j * QBLK:(j + 1) * QBLK],
                                lhsT=kh_sb[b][:, l * S + kt * 128:
                                              l * S + (kt + 1) * 128],
                                rhs=rhs_q, start=True, stop=True)

                    def emit_act(g):
                        n = GRP[g][1]
                        es = es_pool.tile([128, 3 * QBLK], BF16, tag="es",
                                          name=f"es{g}")
                        ess[g] = es
                        nc.scalar.activation(
                            es[:, :n * QBLK], wides[g][:, :n * QBLK],
                            mybir.ActivationFunctionType.Exp, scale=scale)

                    def emit_pv(g):
                        k0, n = GRP[g]
                        for j in range(n):
                            kt = k0 + j
                            nc.tensor.matmul(
                                pv[:],
                                lhsT=vh_sb[b][:, kt * DPC + l * 128:
                                              kt * DPC + (l + 1) * 128],
                                rhs=ess[g][:, j * QBLK:(j + 1) * QBLK],
                                start=(kt == 0), stop=(kt == NKT - 1))

                    def emit_add(g):
                        a = acc_pool.tile([128, 3 * QBLK], BF16, tag="acc",
                                          name=f"acc{g}")
                        if g == 1:
                            nc.vector.tensor_add(a[:], ess[0][:], ess[1][:])
                        else:
                            nc.vector.tensor_add(a[:], accs[g - 1][:],
                                                 ess[g][:])
                        accs[g] = a

                    # software-pipelined emission (1-group lookahead)
                    emit_scores(0)
                    emit_scores(1)
                    emit_act(0)
                    emit_pv(0)
                    for g in range(1, 5):
                        emit_scores(g + 1)
                        emit_act(g)
                        emit_pv(g)
                        emit_add(g)
                    emit_act(5)
                    emit_pv(5)

                    # denominator: fold 1536 -> 512 (+ last group), then
                    # partition reduce + reciprocal + normalize.
                    last = (qb == NQB - 1 and l == HPC - 1)
                    a4 = accs[4]
                    f1 = fld_pool.tile([128, QBLK], BF16, tag="f1")
                    nc.vector.tensor_add(f1[:], a4[:, 0:QBLK],
                                         a4[:, QBLK:2 * QBLK])
                    f2 = fld_pool.tile([128, QBLK], BF16, tag="f2")
                    nc.vector.tensor_add(f2[:], f1[:], a4[:, 2 * QBLK:3 * QBLK])
                    rb = rb_pool.tile([128, QBLK], F32, tag="rb")
                    if last:
                        # short tail: partition-reduce on the PE (ones
                        # matmul into a dead wide slot) instead of the
                        # 3.5us gpsimd reduce — the batch's softmax tail
                        # sits on the next phase's critical path.
                        f3b = fld_pool.tile([128, QBLK], BF16, tag="f3b")
                        nc.vector.tensor_add(f3b[:], f2[:],
                                             ess[5][:, 0:QBLK])
                        rbp = psB_pool.tile([128, 3 * QBLK], F32, tag="w",
                                            name="rbp")
                        nc.tensor.matmul(rbp[:, 0:QBLK], lhsT=ones_sb[:],
                                         rhs=f3b[:], start=True, stop=True)
                        nc.vector.reciprocal_approx_fast(rb[:],
                                                         rbp[:, 0:QBLK])
                    else:
                        f3 = fld_pool.tile([128, QBLK], F32, tag="f3")
                        nc.vector.tensor_add(f3[:], f2[:], ess[5][:, 0:QBLK])
                        nc.gpsimd.partition_all_reduce(
                            rb[:], f3[:], channels=128,
                            reduce_op=bass_isa.ReduceOp.add)
                        nc.vector.reciprocal_approx_fast(rb[:], rb[:])
                    ao = ao_pool.tile([128, QBLK], BF16, tag="ao")
                    nc.vector.tensor_mul(ao[:], pv[:], rb[:])
                    # scatter into a2a_in[b]: chunk m rows [m*DPC+l*128, +128)
                    # on the vector queue — the sync queue must stay free
                    # for the next phase's input rows.
                    q0 = qb * QBLK
                    while q0 < (qb + 1) * QBLK:
                        m = q0 // SCB
                        cend = min((qb + 1) * QBLK, (m + 1) * SCB)
                        nc.gpsimd.dma_start(
                            out=a2a_in[b][m * DPC + l * 128:
                                          m * DPC + (l + 1) * 128,
                                          q0 - m * SCB: cend - m * SCB],
                            in_=ao[:, q0 - qb * QBLK: cend - qb * QBLK])
                        q0 = cend

    cc_tiles = []
    cc_pool = ctx.enter_context(tc.tile_pool(name="cc", bufs=B))

    def fire_a2a(b):
        coll = nc.gpsimd.collective_compute(
            "AllToAll", mybir.AluOpType.bypass,
            replica_groups=[list(range(N_CORES))],
            ins=[a2a_in[b][:, :]], outs=[a2a_out[b][:, :]])
        cc_sb = cc_pool.tile([128, NHT * SCB], BF16, tag="cc", name=f"cc{b}")
        dma = nc.gpsimd.dma_start(
            out=cc_sb[:].rearrange("p (t s) -> p t s", t=NHT),
            in_=a2a_out[b][:, :].rearrange("(t p) s -> p t s", p=128))
        tile.add_dep_helper(dma.ins, coll.ins,
                            reason="a2a_out after collective")
        cc_tiles.append(cc_sb)

    # weights go down the scalar engine's DMA queue so the sync queue
    # carries only input rows (rows must win the pre-barrier bandwidth).
    wq_ck = NHT * DPC // 4
    for ck in range(4):
        nc.scalar.dma_start(out=wq_sb[:, ck * wq_ck:(ck + 1) * wq_ck],
                            in_=wqT[:, ck * wq_ck:(ck + 1) * wq_ck])
    phase_a(0)
    # wo rides the sync queue right after A0's rows: it drains during
    # phase B0 without stealing bandwidth from the startup row stream.
    wo_ck = NHT * HIDDEN // 4
    for ck in range(4):
        nc.sync.dma_start(out=wo_sb[:, ck * wo_ck:(ck + 1) * wo_ck],
                          in_=woT[:, ck * wo_ck:(ck + 1) * wo_ck])
    phase_b(0)
    fire_a2a(0)
    phase_a(1)
    phase_b(1)
    fire_a2a(1)

    # ================= Phase D: output projection (per batch) =========
    with tc.tile_pool(name="pso", bufs=8, space="PSUM") as pso_pool:
        for b in range(B):
            cc_sb = cc_tiles[b]
            for st in range(NST):
                pso = [pso_pool.tile([128, OBLK], F32, tag="pso",
                                     name=f"pso{b}_{st}_{i}")
                       for i in range(NOB)]
                for ht in range(NHT):
                    lhs = cc_sb[:, ht * SCB + st * 128:
                                ht * SCB + (st + 1) * 128]
                    for ot in range(NOB):
                        nc.tensor.matmul(
                            pso[ot][:], lhsT=lhs,
                            rhs=wo_sb[:, ht * HIDDEN + ot * OBLK:
                                      ht * HIDDEN + (ot + 1) * OBLK],
                            start=(ht == 0), stop=(ht == NHT - 1))
                for ot in range(NOB):
                    osb = osb_pool.tile([128, OBLK], F32, tag="osb")
                    nc.vector.tensor_copy(osb[:], pso[ot][:])
                    nc.scalar.dma_start(
                        out=out[b * SCB + st * 128: b * SCB + (st + 1) * 128,
                                ot * OBLK:(ot + 1) * OBLK],
                        in_=osb[:])


def build_nc(S: int):
    nc = bacc.Bacc("TRN2", target_bir_lowering=False, debug=False,
                   enable_asserts=False, num_devices=N_CORES)
    SCB = S // N_CORES
    aps = {
        "qT": [nc.dram_tensor(f"qT{b}", [HIDDEN, S], BF16,
                              kind="ExternalInput").ap() for b in range(B)],
        "kT": [nc.dram_tensor(f"kT{b}", [HIDDEN, S], BF16,
                              kind="ExternalInput").ap() for b in range(B)],
        "vT": [nc.dram_tensor(f"vT{b}", [HIDDEN, S], BF16,
                              kind="ExternalInput").ap() for b in range(B)],
        "wqT": nc.dram_tensor("wqT", [128, NHT * DPC], BF16,
                              kind="ExternalInput").ap(),
        "wkT": nc.dram_tensor("wkT", [128, NHT * DPC], BF16,
                              kind="ExternalInput").ap(),
        "wvT": nc.dram_tensor("wvT", [128, NHT * DPC], BF16,
                              kind="ExternalInput").ap(),
        "woT": nc.dram_tensor("woT", [128, NHT * HIDDEN], BF16,
                              kind="ExternalInput").ap(),
        "out": nc.dram_tensor("out", [B * SCB, HIDDEN], F32,
                              kind="ExternalOutput").ap(),
        "a2a_in": [nc.dram_tensor(f"a2a_in{b}", [N_CORES * DPC, SCB],
                                  BF16).ap() for b in range(B)],
        "a2a_out": [nc.dram_tensor(f"a2a_out{b}", [N_CORES * DPC, SCB],
                                   BF16).ap() for b in range(B)],
    }
    with tile.TileContext(nc) as tc:
        with ExitStack() as ctx:
            _mha_kernel(ctx, tc, aps, S)
    nc.compile()
    return nc


_NC_CACHE: dict = {}


def _tile_weight(w_slice_T):
    """[H, D] -> [128, (H//128)*D] with 128-row tiles laid out consecutively."""
    H, D = w_slice_T.shape
    return np.ascontiguousarray(
        w_slice_T.reshape(H // 128, 128, D).transpose(1, 0, 2).reshape(
            128, (H // 128) * D))


def make_in_maps(q, k, v, w_q, w_k, w_v, w_o):
    """Host-side shard/cast. Returns per-core input dicts."""
    qT = [np.ascontiguousarray(q[b].T).astype(NPBF16) for b in range(B)]
    kT = [np.ascontiguousarray(k[b].T).astype(NPBF16) for b in range(B)]
    vT = [np.ascontiguousarray(v[b].T).astype(NPBF16) for b in range(B)]
    woT = _tile_weight(np.ascontiguousarray(w_o.T).astype(NPBF16))
    in_maps = []
    for c in range(N_CORES):
        d0 = c * DPC
        m = {}
        for b in range(B):
            m[f"qT{b}"] = qT[b]
            m[f"kT{b}"] = kT[b]
            m[f"vT{b}"] = vT[b]
        m["wqT"] = _tile_weight(
            np.ascontiguousarray(w_q[d0:d0 + DPC, :].T).astype(NPBF16))
        m["wkT"] = _tile_weight(
            np.ascontiguousarray(w_k[d0:d0 + DPC, :].T).astype(NPBF16))
        m["wvT"] = _tile_weight(
            np.ascontiguousarray(w_v[d0:d0 + DPC, :].T).astype(NPBF16))
        m["woT"] = woT
        in_maps.append(m)
    return in_maps


def kernel(q, k, v, mask, w_q, w_k, w_v, w_o, _trace=False):
    q = np.asarray(q, np.float32)
    k = np.asarray(k, np.float32)
    v = np.asarray(v, np.float32)
    mask = np.asarray(mask)
    w_q = np.asarray(w_q, np.float32)
    w_k = np.asarray(w_k, np.float32)
    w_v = np.asarray(w_v, np.float32)
    w_o = np.asarray(w_o, np.float32)
    S = q.shape[1]

    if not np.all(mask != 0):
        # General-mask fallback (never hit for the eval problem: mask is
        # all ones).  Computed on host for correctness.
        return _numpy_reference(q, k, v, mask, w_q, w_k, w_v, w_o)

    if S not in _NC_CACHE:
        _NC_CACHE[S] = build_nc(S)
    nc = _NC_CACHE[S]

    in_maps = make_in_maps(q, k, v, w_q, w_k, w_v, w_o)
    res = run_bass_kernel_spmd(nc, in_maps, core_ids=list(range(N_CORES)),
                               trace=_trace)

    SCB = S // N_CORES
    out = np.empty((B, S, HIDDEN), np.float32)
    for c in range(N_CORES):
        for b in range(B):
            out[b, c * SCB:(c + 1) * SCB, :] = \
                res.results[c]["out"][b * SCB:(b + 1) * SCB, :]
    if _trace:
        return out, res
    return out


def _numpy_reference(q, k, v, mask, w_q, w_k, w_v, w_o):
    Bn, S, H = q.shape
    dk = H // HEADS

    def split_heads(x, w):
        y = x @ w.T
        return y.reshape(Bn, S, HEADS, dk).transpose(0, 2, 1, 3)

    qh = split_heads(q, w_q)
    kh = split_heads(k, w_k)
    vh = split_heads(v, w_v)
    s = np.einsum("bhqd,bhkd->bhqk", qh, kh) / np.sqrt(np.float32(dk))
    s = np.where(mask[:, None, :, :] == 0, np.float32(-1e9), s)
    s = s - s.max(-1, keepdims=True)
    e = np.exp(s)
    a = e / e.sum(-1, keepdims=True)
    o = np.einsum("bhqk,bhkd->bhqd", a, vh)
    o = o.transpose(0, 2, 1, 3).reshape(Bn, S, H)
    return (o @ w_o.T).astype(np.float32)
